# revision 20
# baseline (speedup 1.0000x reference)
"""Trainium2 kernel for DetContrastiveLoss (embedding_lookup).

Reference semantics (buggy original preserved): only the LAST batch element of
spatial_features_2d / gt_boxes is used.  500 box centers are mapped to pixel
indices, the 128-channel feature vector at each pixel is gathered from the
128 MB feature map resident in device HBM, L2-normalized, and a 500x500
cosine-similarity contrastive loss (log_softmax + label pick) is reduced to a
scalar.

Single-launch, single-core design (v2 of this problem; v1 used two launches
and measured 66 us -- most of it was a second NEFF preamble, a host
round-trip between passes, and a [128,1] column DMA that cost 6.4 us in
descriptor spray):

  - The full pixel-major table [H*W, C] lives in HBM.  gpsimd issues 4
    indirect DMAs (128 indices each, one 512 B channel row per pixel) into a
    pixel-major SBUF tile [128, 4*128].
  - Per tile, a 5-stage pipeline across engines: DVE tensor_tensor_reduce
    gives per-pixel sum-of-squares [128,1]; ACT computes 1/norm as
    exp(-0.5*ln(css)) (Ln and Exp share one activation table --
    natural_log_exp_and_others -- so after a dummy-exp prefetch there are no
    table loads on the critical path; Rsqrt is banned in bass); DVE scales
    the raw tile by 1/norm (per-partition scalar); PE transposes the
    normalized tile (identity comes in as an input DMA); ACT copies PSUM ->
    SBUF.
  - PE computes the Gram matrix as 4 matmuls [128, 500] (lhsT = 128-column
    block of the normalized channel-major features).  ACT exponentiates each
    block with scale=1/T and accumulates row sums (softmax denominator)
    for free; DVE picks the labeled logit per row with a fused
    mask-multiply-reduce against a HOST-precomputed one-hot label mask.
  - val = pick - ln(ssum) rows are dotted with a validity mask and reduced
    across partitions by a [1,1] matmul; ACT copies the scalar out of PSUM
    and issues the 4-byte output DMA itself (avoiding a cross-engine hop and
    the column-DMA descriptor spray).

Host does: pixel index math (exact fp32 replication), one-hot label mask,
table transpose to pixel-major, and the final -LOSS_SCALE/N scaling.
"""

import contextlib
import ctypes
import os
import sys
import types

import numpy as np

from concourse import bass, mybir
from concourse import bass_utils as _bass_utils
from concourse.bass_utils import run_bass_kernel_spmd

# Problem geometry (hardcoded per spec nn_DetContrastiveLoss_72636486910298).
B, C, H, W = 4, 128, 512, 512
HW = H * W
N = 500
NT = 4                  # pixel tiles of 128 (500 padded to 512)
NPAD = NT * 128

PC_RANGE = (-59.9, -59.9, -2.0, 59.9, 59.9, 5.9)
TEMPERATURE = 0.1
LOSS_SCALE = 0.01
INV_T = float(1.0 / TEMPERATURE)

F32 = mybir.dt.float32
I32 = mybir.dt.int32

# Observability for test.py: exec_time_ns of the last run (populated only when
# KERNEL_TRACE=1 so profiling is on).
LAST_EXEC_NS = {"main": None}
LAST_TRACE_DIRS = {"main": None}
_DEBUG = {}


def _install_ntff_hook():
    """Provide antenv.axon_hooks (absent in this image) so bass_utils'
    trace=True path can capture NTFF profiles via the axon PJRT .so."""
    try:
        import antenv.axon_hooks  # noqa: F401
        return
    except ImportError:
        pass
    hook = None
    so_path = "/opt/axon/libaxon_pjrt.so"
    if os.path.exists(so_path):
        lib = ctypes.CDLL(so_path)
        if hasattr(lib, "axon_start_nrt_profile"):
            lib.axon_start_nrt_profile.argtypes = [
                ctypes.POINTER(ctypes.c_int64), ctypes.c_size_t]
            lib.axon_start_nrt_profile.restype = ctypes.c_int64
            lib.axon_stop_nrt_profile.argtypes = [ctypes.c_char_p]
            lib.axon_stop_nrt_profile.restype = ctypes.c_int64

            @contextlib.contextmanager
            def _hook(output_dir, device_ids):
                import jax
                jax.devices()
                if device_ids:
                    ids = (ctypes.c_int64 * len(device_ids))(*device_ids)
                    rc = lib.axon_start_nrt_profile(ids, len(device_ids))
                else:
                    rc = lib.axon_start_nrt_profile(None, 0)
                if rc != 0:
                    raise RuntimeError(f"axon_start_nrt_profile rc={rc}")
                try:
                    yield
                finally:
                    n = lib.axon_stop_nrt_profile(str(output_dir).encode())
                    print(f"profile: {n} file(s) -> {output_dir}", file=sys.stderr)

            hook = _hook
    mod = types.ModuleType("antenv.axon_hooks")
    mod.get_axon_ntff_profile_hook = lambda: hook
    mod.set_axon_ntff_profile_hook = lambda h: None
    sys.modules["antenv.axon_hooks"] = mod


def _run(prog_key, in_maps, core_ids):
    """run_bass_kernel_spmd with env-gated tracing."""
    progs = _progs()
    if os.environ.get("KERNEL_TRACE"):
        _install_ntff_hook()
        # Artifact upload needs network egress; keep everything local.
        _bass_utils.upload_artifacts = lambda tmpdir: "local://" + str(tmpdir)
        import tempfile
        tmpdir = tempfile.mkdtemp(prefix=f"bass_{prog_key}_")
        LAST_TRACE_DIRS[prog_key] = tmpdir
        res = run_bass_kernel_spmd(
            progs[prog_key], in_maps, core_ids=core_ids,
            trace=True, tmpdir=tmpdir,
        )
    else:
        # Never let a stray BASS_TRACE in the environment route us into the
        # trace path (its antenv import may be unavailable).
        old = os.environ.get("BASS_NEVER_TRACE")
        os.environ["BASS_NEVER_TRACE"] = "1"
        try:
            res = run_bass_kernel_spmd(progs[prog_key], in_maps,
                                       core_ids=core_ids)
        finally:
            if old is None:
                os.environ.pop("BASS_NEVER_TRACE", None)
            else:
                os.environ["BASS_NEVER_TRACE"] = old
    LAST_EXEC_NS[prog_key] = res.exec_time_ns
    return res


def _build_main_prog(stage="full"):
    # stage: debug knob -- "front" stops after the rhsN copies and DMAs
    # rhsN out; "gram" adds G0 matmuls + ssum exps and DMAs sspk out;
    # "full" is the real kernel.
    # Raw bass (no TileContext): the trimmed walrus pipeline here can't
    # codegen Tile's tail drain, and raw bass also skips the exit barrier.
    #
    # HW indirect-DMA semantics (verified on device in the v1 session): each
    # index addresses a CONTIGUOUS run of rowsize elements at
    # table_flat[idx*coef], one index per dest partition; source AP strides
    # are not honored.  Table is pixel-major [HW, C]; one index gathers one
    # pixel's full 512 B channel row.
    nc = bass.Bass("TRN2", target_bir_lowering=False)
    table = nc.dram_tensor("table", [HW, C], F32, kind="ExternalInput")
    idx_d = nc.dram_tensor("idx", [128, NT], I32, kind="ExternalInput")
    ident_d = nc.dram_tensor("ident", [128, 128], F32, kind="ExternalInput")
    lmask_d = nc.dram_tensor("lmask", [128, NT * N], F32, kind="ExternalInput")
    rmask_d = nc.dram_tensor("rmask", [128, NT], F32, kind="ExternalInput")
    out_d = nc.dram_tensor("partial", [1, 1], F32, kind="ExternalOutput")
    dbg_rhs_d = (nc.dram_tensor("dbg_rhs", [128, NPAD], F32, kind="ExternalOutput")
                 if stage == "front" else None)
    dbg_ssum_d = (nc.dram_tensor("dbg_ssum", [128, NT], F32,
                                 kind="ExternalOutput")
                  if stage == "gram" else None)

    AF = mybir.ActivationFunctionType
    OP = mybir.AluOpType
    F32R = mybir.dt.float32r

    from contextlib import ExitStack
    with ExitStack() as ctx:
        def sb(name, shape, dt=F32):
            return ctx.enter_context(nc.sbuf_tensor(name, shape, dt))
        idx_sb = sb("idx_sb", [128, NT], I32)
        I_sb = sb("I_sb", [128, 128])
        lmask_sb = sb("lmask_sb", [128, NT * N])
        rmask_sb = sb("rmask_sb", [128, NT])
        traw = sb("traw", [128, NPAD])    # gathered pixel-major raw features
        trawN = sb("trawN", [128, NPAD])  # per-pixel normalized features
        # float32r: PE runs 1 cycle/row (vs 4 for fp32); the ACT copies
        # below perform the required fp32r rounding on write.
        rhsN = sb("rhsN", [128, NPAD], F32R)  # channel-major normalized
        sqscr = sb("sqscr", [128, 128])   # ACT square scratch (unused)
        css = sb("css", [128, NT])        # per-pixel sum of squares
        lncss = sb("lncss", [128, NT])
        inv = sb("inv", [128, NT])        # 1/norm per pixel
        escrs = [sb(f"escr{m}", [128, N]) for m in range(NT)]  # exp(sim/T)
        mscrs = [sb(f"mscr{m}", [128, N]) for m in range(NT)]  # pick products
        sspk = sb("sspk", [128, 2 * NT])  # cols 0:NT softmax denominators,
                                          # cols NT:2NT picked exp(logit)
        lnall = sb("lnall", [128, 2 * NT])
        val = sb("val", [128, NT])
        vscr = sb("vscr", [128, NT])
        tmp = sb("tmp", [128, 1])
        ones_r = sb("ones_r", [128, 1])
        scr0 = sb("scr0", [1, 1])
        scr1 = sb("scr1", [1, 1])
        res = sb("res", [1, 1])
        # HW quirk (bisected): 4 transpose matmuls into one full PSUM bank
        # fault the NEFF; 2x2 across two half-banks is fine.
        Thats = [ctx.enter_context(nc.psum_tensor(f"That{i}", [128, 256], F32))
                 for i in range(2)]
        G0 = [ctx.enter_context(nc.psum_tensor(f"G0_{m}", [128, N], F32))
              for m in range(NT)]
        tot = ctx.enter_context(nc.psum_tensor("tot", [1, 1], F32))
        # One semaphore per DMA milestone: transfers sharing a queue
        # complete out-of-order across the 16 SDMA engines, so a shared
        # counter's intermediate values are not sound sync points (CoreSim
        # SemaphoreRace caught this).
        sidx = ctx.enter_context(nc.semaphore())
        sid = ctx.enter_context(nc.semaphore())
        smsk = ctx.enter_context(nc.semaphore())
        sg = [ctx.enter_context(nc.semaphore(name=f"sg{t}"))
              for t in range(NT)]
        smul = ctx.enter_context(nc.semaphore(name="smul"))
        vsem = ctx.enter_context(nc.semaphore())
        asem = ctx.enter_context(nc.semaphore())
        psem = ctx.enter_context(nc.semaphore())
        osem = ctx.enter_context(nc.semaphore())
        block = ctx.enter_context(nc.Block())

        # Raw-bass hazard discipline (v1 lesson): compute-engine sem updates
        # can fire before the data write lands, so every release that another
        # engine (or a dependent same-engine op) consumes rides on a drain()
        # of the producing engine.  DMA completion increments are safe as-is.

        @block.gpsimd
        def _(g):
            # All DMAs on gpsimd's qPoolDynamic: the SP/ACT HW-DGE rings
            # error out on this runtime (bisected on HW).
            g.dma_start(idx_sb[:], idx_d[:]).then_inc(sidx, 16)
            g.dma_start(I_sb[:], ident_d[:]).then_inc(sid, 16)
            g.dma_start(lmask_sb[:], lmask_d[:]).then_inc(smsk, 16)
            g.dma_start(rmask_sb[:], rmask_d[:]).then_inc(smsk, 16)
            g.wait_ge(sidx, 16)
            for t in range(NT):
                g.indirect_dma_start(
                    out=traw[:, t * 128:(t + 1) * 128],
                    out_offset=None,
                    in_=table[:],
                    in_offset=bass.IndirectOffsetOnAxis(
                        ap=idx_sb[:, t:t + 1], axis=0),
                ).then_inc(sg[t], 16)
            if stage == "full":
                g.wait_ge(asem, 15)     # res holds the scalar
                g.dma_start(out_d[:], res[:]).then_inc(osem, 16)
                g.wait_ge(osem, 16)
            if stage == "front":
                g.wait_ge(asem, 9)
                g.dma_start(dbg_rhs_d[:], rhsN[:].bitcast(F32)).then_inc(osem, 16)
                g.wait_ge(osem, 16)
                return

        @block.vector
        def _(v):
            v.memset(scr0[:], 0.0)
            v.memset(ones_r[:], 1.0)
            v.drain().then_inc(vsem, 1)                              # v1
            for t in range(NT):
                tile = slice(t * 128, (t + 1) * 128)
                v.wait_ge(asem, 2 + t)
                v.tensor_scalar_mul(
                    out=trawN[:, tile], in0=traw[:, tile],
                    scalar1=inv[:, t:t + 1])
                v.drain().then_inc(vsem, 1)                          # v2+t
            if stage != "full":
                return
            v.wait_ge(smsk, 32)     # label mask landed
            for m in range(NT):
                v.wait_ge(asem, 10 + m)
                v.tensor_mul(mscrs[m][:], escrs[m][:],
                             lmask_sb[:, m * N:(m + 1) * N])
                v.drain()
                v.reduce_sum(out=sspk[:, NT + m:NT + m + 1],
                             in_=mscrs[m][:], axis=mybir.AxisListType.X)
                v.drain().then_inc(vsem, 1)                          # v6+m
            v.wait_ge(asem, 14)     # lnall ready
            # val = ln(exp(logit_pick)) - ln(sum exp) per row
            v.tensor_sub(val[:], lnall[:, NT:2 * NT], lnall[:, 0:NT])
            v.drain()
            v.wait_ge(smsk, 32)     # row-validity mask landed
            v.tensor_mul(vscr[:], val[:], rmask_sb[:])
            v.drain()
            v.reduce_sum(out=tmp[:], in_=vscr[:], axis=mybir.AxisListType.X)
            v.drain().then_inc(vsem, 1)                              # v10

        @block.scalar
        def _(a):
            a.wait_ge(vsem, 1)
            # Dummy exp: triggers the (single) activation table load during
            # the idx-DMA latency window instead of on the critical path.
            a.activation(out=scr1[:], in_=scr0[:], func=AF.Exp)
            a.drain().then_inc(asem, 1)                              # a1
            for t in range(NT):
                tile = slice(t * 128, (t + 1) * 128)
                a.wait_ge(sg[t], 16)
                # sum-of-squares per pixel comes free from the Square
                # activation's row accumulator
                a.activation(out=sqscr[:], in_=traw[:, tile], func=AF.Square,
                             accum_out=css[:, t:t + 1])
                a.drain()
                a.activation(out=lncss[:, t:t + 1], in_=css[:, t:t + 1],
                             func=AF.Ln)
                a.drain()
                # 1/norm = exp(-0.5 * ln(css))
                a.activation(out=inv[:, t:t + 1], in_=lncss[:, t:t + 1],
                             func=AF.Exp, scale=-0.5)
                a.drain().then_inc(asem, 1)                          # a2+t
            for t in range(NT):
                tile = slice(t * 128, (t + 1) * 128)
                a.wait_ge(psem, 1 + t)
                a.copy(out=rhsN[:, tile],
                       in_=Thats[t // 2][:, (t % 2) * 128:(t % 2 + 1) * 128])
                a.drain().then_inc(asem, 1)                          # a6+t
            if stage == "front":
                return
            for m in range(NT):
                a.wait_ge(psem, 5 + m)
                a.activation(out=escrs[m][:], in_=G0[m][:, 0:N], func=AF.Exp,
                             scale=INV_T, accum_out=sspk[:, m:m + 1])
                a.drain().then_inc(asem, 1)                          # a10+m
            if stage == "gram":
                a.dma_start(dbg_ssum_d[:], sspk[:, 0:NT]).then_inc(osem, 16)
                a.wait_ge(osem, 16)
                return
            a.wait_ge(vsem, 9)      # picked-exp reductions landed
            a.activation(out=lnall[:], in_=sspk[:], func=AF.Ln)
            a.drain().then_inc(asem, 1)                              # a14
            a.wait_ge(psem, 9)
            a.mul(res[:], tot[:], 1.0)
            a.drain().then_inc(asem, 1)                              # a15

        @block.tensor
        def _(te):
            te.wait_ge(sid, 16)     # identity landed
            for t in range(NT):
                tile = slice(t * 128, (t + 1) * 128)
                te.wait_ge(vsem, 2 + t)
                nc.tensor.transpose(
                    Thats[t // 2][:, (t % 2) * 128:(t % 2 + 1) * 128],
                    trawN[:, tile], I_sb[:])
                te.drain().then_inc(psem, 1)                         # p1+t
            if stage == "front":
                return
            te.wait_ge(asem, 9)     # all PSUM->SBUF copies done
            for m in range(NT):
                # float32r runs the PE at 1 cycle/row (vs 4 for fp32); the
                # tolerance (2e-2) dwarfs the precision delta.
                nc.tensor.matmul(
                    G0[m][:, 0:N], lhsT=rhsN[:, m * 128:(m + 1) * 128],
                    rhs=rhsN[:, 0:N], start=True, stop=True)
                te.drain().then_inc(psem, 1)                         # p5+m
            if stage == "gram":
                return
            te.wait_ge(vsem, 10)
            nc.tensor.matmul(tot[:], lhsT=tmp[:], rhs=ones_r[:],
                             start=True, stop=True)
            te.drain().then_inc(psem, 1)                             # p9
    return nc


_PROGS = {}


def _progs():
    if not _PROGS:
        _PROGS["main"] = _build_main_prog()
    return _PROGS


def _pixel_indices(gt_boxes: np.ndarray) -> np.ndarray:
    """Exact fp32 replication of the reference pixel-index math (last batch)."""
    boxes = np.asarray(gt_boxes)[B - 1].astype(np.float32, copy=False)
    x = boxes[:, 0].astype(np.float32)
    y = boxes[:, 1].astype(np.float32)
    span_w = PC_RANGE[3] - PC_RANGE[0]
    span_h = PC_RANGE[4] - PC_RANGE[1]
    cx = (x - np.float32(PC_RANGE[0])) / np.float32(span_w) * np.float32(W)
    cy = (y - np.float32(PC_RANGE[1])) / np.float32(span_h) * np.float32(H)
    cx = np.clip(cx.astype(np.int32), 0, W - 1)
    cy = np.clip(cy.astype(np.int32), 0, H - 1)
    return (cy.astype(np.int64) * W + cx.astype(np.int64)).astype(np.int32)


def kernel(spatial_features_2d, gt_boxes, static_labels, dynamic_labels,
           num_static=None, **_unused):
    sf = np.asarray(spatial_features_2d)
    pix = _pixel_indices(gt_boxes)  # [N] int32, linear index into H*W plane

    # Pixel-major table: one 512 B contiguous channel row per pixel.
    table = np.ascontiguousarray(
        sf[B - 1].reshape(C, HW).T, dtype=np.float32)   # [HW, C]

    pix_pad = np.zeros(NPAD, dtype=np.int32)
    pix_pad[:N] = pix
    idx = np.ascontiguousarray(pix_pad.reshape(NT, 128).T)  # [128, NT]

    labels = np.concatenate(
        [np.asarray(static_labels), np.asarray(dynamic_labels)], axis=0
    ).astype(np.int64)
    # One-hot label mask, row-block-major: block m in columns [m*N,(m+1)*N).
    lmask = np.zeros((128, NT * N), dtype=np.float32)
    rmask = np.zeros((128, NT), dtype=np.float32)
    for r in range(NT * 128):
        m, p = divmod(r, 128)
        if r < N:
            lmask[p, m * N + int(labels[r])] = 1.0
            rmask[p, m] = 1.0
        else:
            # invalid rows still need a finite ln(picked-exp); rmask zeroes
            # their contribution, but 0*ln(0) would be NaN
            lmask[p, m * N] = 1.0

    ident = np.eye(128, dtype=np.float32)

    in_maps = [{
        "table": table,
        "idx": idx,
        "ident": ident,
        "lmask": lmask,
        "rmask": rmask,
    }]
    r = _run("main", in_maps, core_ids=[0])
    total = float(r.results[0]["partial"][0, 0])
    _DEBUG["total"] = total
    loss = np.float32(total * (-LOSS_SCALE / N))
    return np.array(loss, dtype=np.float32)


# revision 23
# speedup vs baseline: 1.0310x; 1.0310x over previous
"""Trainium2 kernel for DetContrastiveLoss (embedding_lookup).

Reference semantics (buggy original preserved): only the LAST batch element of
spatial_features_2d / gt_boxes is used.  500 box centers are mapped to pixel
indices, the 128-channel feature vector at each pixel is gathered from the
128 MB feature map resident in device HBM, L2-normalized, and a 500x500
cosine-similarity contrastive loss (log_softmax + label pick) is reduced to a
scalar.

Single-launch, single-core design (v2 of this problem; v1 used two launches
and measured 66 us -- most of it was a second NEFF preamble, a host
round-trip between passes, and a [128,1] column DMA that cost 6.4 us in
descriptor spray):

  - The full pixel-major table [H*W, C] lives in HBM.  gpsimd issues 4
    indirect DMAs (128 indices each, one 512 B channel row per pixel) into a
    pixel-major SBUF tile [128, 4*128].
  - Per tile, a 5-stage pipeline across engines: DVE tensor_tensor_reduce
    gives per-pixel sum-of-squares [128,1]; ACT computes 1/norm as
    exp(-0.5*ln(css)) (Ln and Exp share one activation table --
    natural_log_exp_and_others -- so after a dummy-exp prefetch there are no
    table loads on the critical path; Rsqrt is banned in bass); DVE scales
    the raw tile by 1/norm (per-partition scalar); PE transposes the
    normalized tile (identity comes in as an input DMA); ACT copies PSUM ->
    SBUF.
  - PE computes the Gram matrix as 4 matmuls [128, 500] (lhsT = 128-column
    block of the normalized channel-major features).  ACT exponentiates each
    block with scale=1/T and accumulates row sums (softmax denominator)
    for free; DVE picks the labeled logit per row with a fused
    mask-multiply-reduce against a HOST-precomputed one-hot label mask.
  - val = pick - ln(ssum) rows are dotted with a validity mask and reduced
    across partitions by a [1,1] matmul; ACT copies the scalar out of PSUM
    and issues the 4-byte output DMA itself (avoiding a cross-engine hop and
    the column-DMA descriptor spray).

Host does: pixel index math (exact fp32 replication), one-hot label mask,
table transpose to pixel-major, and the final -LOSS_SCALE/N scaling.
"""

import contextlib
import ctypes
import os
import sys
import types

import numpy as np

from concourse import bass, mybir
from concourse import bass_utils as _bass_utils
from concourse.bass_utils import run_bass_kernel_spmd

# Problem geometry (hardcoded per spec nn_DetContrastiveLoss_72636486910298).
B, C, H, W = 4, 128, 512, 512
HW = H * W
N = 500
NT = 4                  # pixel tiles of 128 (500 padded to 512)
NPAD = NT * 128

PC_RANGE = (-59.9, -59.9, -2.0, 59.9, 59.9, 5.9)
TEMPERATURE = 0.1
LOSS_SCALE = 0.01
INV_T = float(1.0 / TEMPERATURE)

F32 = mybir.dt.float32
I32 = mybir.dt.int32

# Observability for test.py: exec_time_ns of the last run (populated only when
# KERNEL_TRACE=1 so profiling is on).
LAST_EXEC_NS = {"main": None}
LAST_TRACE_DIRS = {"main": None}
_DEBUG = {}


def _install_ntff_hook():
    """Provide antenv.axon_hooks (absent in this image) so bass_utils'
    trace=True path can capture NTFF profiles via the axon PJRT .so."""
    try:
        import antenv.axon_hooks  # noqa: F401
        return
    except ImportError:
        pass
    hook = None
    so_path = "/opt/axon/libaxon_pjrt.so"
    if os.path.exists(so_path):
        lib = ctypes.CDLL(so_path)
        if hasattr(lib, "axon_start_nrt_profile"):
            lib.axon_start_nrt_profile.argtypes = [
                ctypes.POINTER(ctypes.c_int64), ctypes.c_size_t]
            lib.axon_start_nrt_profile.restype = ctypes.c_int64
            lib.axon_stop_nrt_profile.argtypes = [ctypes.c_char_p]
            lib.axon_stop_nrt_profile.restype = ctypes.c_int64

            @contextlib.contextmanager
            def _hook(output_dir, device_ids):
                import jax
                jax.devices()
                if device_ids:
                    ids = (ctypes.c_int64 * len(device_ids))(*device_ids)
                    rc = lib.axon_start_nrt_profile(ids, len(device_ids))
                else:
                    rc = lib.axon_start_nrt_profile(None, 0)
                if rc != 0:
                    raise RuntimeError(f"axon_start_nrt_profile rc={rc}")
                try:
                    yield
                finally:
                    n = lib.axon_stop_nrt_profile(str(output_dir).encode())
                    print(f"profile: {n} file(s) -> {output_dir}", file=sys.stderr)

            hook = _hook
    mod = types.ModuleType("antenv.axon_hooks")
    mod.get_axon_ntff_profile_hook = lambda: hook
    mod.set_axon_ntff_profile_hook = lambda h: None
    sys.modules["antenv.axon_hooks"] = mod


def _run(prog_key, in_maps, core_ids):
    """run_bass_kernel_spmd with env-gated tracing."""
    progs = _progs()
    if os.environ.get("KERNEL_TRACE"):
        _install_ntff_hook()
        # Artifact upload needs network egress; keep everything local.
        _bass_utils.upload_artifacts = lambda tmpdir: "local://" + str(tmpdir)
        import tempfile
        tmpdir = tempfile.mkdtemp(prefix=f"bass_{prog_key}_")
        LAST_TRACE_DIRS[prog_key] = tmpdir
        res = run_bass_kernel_spmd(
            progs[prog_key], in_maps, core_ids=core_ids,
            trace=True, tmpdir=tmpdir,
        )
    else:
        # Never let a stray BASS_TRACE in the environment route us into the
        # trace path (its antenv import may be unavailable).
        old = os.environ.get("BASS_NEVER_TRACE")
        os.environ["BASS_NEVER_TRACE"] = "1"
        try:
            res = run_bass_kernel_spmd(progs[prog_key], in_maps,
                                       core_ids=core_ids)
        finally:
            if old is None:
                os.environ.pop("BASS_NEVER_TRACE", None)
            else:
                os.environ["BASS_NEVER_TRACE"] = old
    LAST_EXEC_NS[prog_key] = res.exec_time_ns
    return res


def _build_main_prog(stage="full"):
    # stage: debug knob -- "front" stops after the rhsN copies and DMAs
    # rhsN out; "gram" adds G0 matmuls + ssum exps and DMAs sspk out;
    # "full" is the real kernel.
    # Raw bass (no TileContext): the trimmed walrus pipeline here can't
    # codegen Tile's tail drain, and raw bass also skips the exit barrier.
    #
    # HW indirect-DMA semantics (verified on device in the v1 session): each
    # index addresses a CONTIGUOUS run of rowsize elements at
    # table_flat[idx*coef], one index per dest partition; source AP strides
    # are not honored.  Table is pixel-major [HW, C]; one index gathers one
    # pixel's full 512 B channel row.
    nc = bass.Bass("TRN2", target_bir_lowering=False)
    table = nc.dram_tensor("table", [HW, C], F32, kind="ExternalInput")
    idx_d = nc.dram_tensor("idx", [128, NT], I32, kind="ExternalInput")
    ident_d = nc.dram_tensor("ident", [128, 128], F32, kind="ExternalInput")
    lmask_d = nc.dram_tensor("lmask", [128, NT * N], F32, kind="ExternalInput")
    rmask_d = nc.dram_tensor("rmask", [128, NT], F32, kind="ExternalInput")
    out_d = nc.dram_tensor("partial", [1, 1], F32, kind="ExternalOutput")
    dbg_rhs_d = (nc.dram_tensor("dbg_rhs", [128, NPAD], F32, kind="ExternalOutput")
                 if stage == "front" else None)
    dbg_ssum_d = (nc.dram_tensor("dbg_ssum", [128, NT], F32,
                                 kind="ExternalOutput")
                  if stage == "gram" else None)

    AF = mybir.ActivationFunctionType
    OP = mybir.AluOpType
    F32R = mybir.dt.float32r

    from contextlib import ExitStack
    with ExitStack() as ctx:
        def sb(name, shape, dt=F32):
            return ctx.enter_context(nc.sbuf_tensor(name, shape, dt))
        idx_sb = sb("idx_sb", [128, NT], I32)
        I_sb = sb("I_sb", [128, 128])
        lmask_sb = sb("lmask_sb", [128, NT * N])
        rmask_sb = sb("rmask_sb", [128, NT])
        traw = sb("traw", [128, NPAD])    # gathered pixel-major raw features
        trawN = sb("trawN", [128, NPAD])  # per-pixel normalized features
        # float32r: PE runs 1 cycle/row (vs 4 for fp32); the ACT copies
        # below perform the required fp32r rounding on write.
        rhsN = sb("rhsN", [128, NPAD], F32R)  # channel-major normalized
        sqscr = sb("sqscr", [128, 128])   # ACT square scratch (unused)
        css = sb("css", [128, NT])        # per-pixel sum of squares
        lncss = sb("lncss", [128, NT])
        inv = sb("inv", [128, NT])        # 1/norm per pixel
        escrs = [sb(f"escr{m}", [128, N]) for m in range(NT)]  # exp(sim/T)
        mscrs = [sb(f"mscr{m}", [128, N]) for m in range(NT)]  # pick products
        sspk = sb("sspk", [128, 2 * NT])  # cols 0:NT softmax denominators,
                                          # cols NT:2NT picked exp(logit)
        lnall = sb("lnall", [128, 2 * NT])
        val = sb("val", [128, NT])
        vscr = sb("vscr", [128, NT])
        tmp = sb("tmp", [128, 1])
        ones_r = sb("ones_r", [128, 1])
        scr0 = sb("scr0", [1, 1])
        scr1 = sb("scr1", [1, 1])
        res = sb("res", [1, 1])
        # HW quirk (bisected): 4 transpose matmuls into one full PSUM bank
        # fault the NEFF; 2x2 across two half-banks is fine.
        Thats = [ctx.enter_context(nc.psum_tensor(f"That{i}", [128, 256], F32))
                 for i in range(2)]
        G0 = [ctx.enter_context(nc.psum_tensor(f"G0_{m}", [128, N], F32))
              for m in range(NT)]
        tot = ctx.enter_context(nc.psum_tensor("tot", [1, 1], F32))
        # One semaphore per DMA milestone: transfers sharing a queue
        # complete out-of-order across the 16 SDMA engines, so a shared
        # counter's intermediate values are not sound sync points (CoreSim
        # SemaphoreRace caught this).
        sidx = ctx.enter_context(nc.semaphore())
        sid = ctx.enter_context(nc.semaphore())
        smsk = ctx.enter_context(nc.semaphore())
        sg = [ctx.enter_context(nc.semaphore(name=f"sg{t}"))
              for t in range(NT)]
        smul = ctx.enter_context(nc.semaphore(name="smul"))
        vsem = ctx.enter_context(nc.semaphore())
        asem = ctx.enter_context(nc.semaphore())
        psem = ctx.enter_context(nc.semaphore())
        osem = ctx.enter_context(nc.semaphore())
        block = ctx.enter_context(nc.Block())

        # Raw-bass hazard discipline (v1 lesson): compute-engine sem updates
        # can fire before the data write lands, so every release that another
        # engine (or a dependent same-engine op) consumes rides on a drain()
        # of the producing engine.  DMA completion increments are safe as-is.

        @block.gpsimd
        def _(g):
            # All DMAs on gpsimd's qPoolDynamic: the SP/ACT HW-DGE rings
            # error out on this runtime (bisected on HW).
            g.dma_start(idx_sb[:], idx_d[:]).then_inc(sidx, 16)
            g.dma_start(I_sb[:], ident_d[:]).then_inc(sid, 16)
            g.wait_ge(sidx, 16)
            for t in range(NT):
                g.indirect_dma_start(
                    out=traw[:, t * 128:(t + 1) * 128],
                    out_offset=None,
                    in_=table[:],
                    in_offset=bass.IndirectOffsetOnAxis(
                        ap=idx_sb[:, t:t + 1], axis=0),
                ).then_inc(sg[t], 16)
            # the 1 MB label mask rides after the gathers so its transfer
            # doesn't contend with them; it's not needed until pick time
            g.dma_start(lmask_sb[:], lmask_d[:]).then_inc(smsk, 16)
            g.dma_start(rmask_sb[:], rmask_d[:]).then_inc(smsk, 16)
            if stage == "full":
                g.wait_ge(vsem, 10)     # early wake: tmp underway
                g.wait_ge(asem, 15)     # res holds the scalar
                g.dma_start(out_d[:], res[:]).then_inc(osem, 16)
                g.wait_ge(osem, 16)
            if stage == "front":
                g.wait_ge(asem, 9)
                g.dma_start(dbg_rhs_d[:], rhsN[:].bitcast(F32)).then_inc(osem, 16)
                g.wait_ge(osem, 16)
                return

        @block.vector
        def _(v):
            v.memset(scr0[:], 0.0)
            v.memset(ones_r[:], 1.0)
            v.drain().then_inc(vsem, 1)                              # v1
            for t in range(NT):
                tile = slice(t * 128, (t + 1) * 128)
                v.wait_ge(asem, 2 + t)
                v.tensor_scalar_mul(
                    out=trawN[:, tile], in0=traw[:, tile],
                    scalar1=inv[:, t:t + 1])
                v.drain().then_inc(vsem, 1)                          # v2+t
            if stage != "full":
                return
            v.wait_ge(smsk, 32)     # label mask landed
            for m in range(NT):
                v.wait_ge(asem, 10 + m)
                v.tensor_mul(mscrs[m][:], escrs[m][:],
                             lmask_sb[:, m * N:(m + 1) * N])
                v.drain()
                v.reduce_sum(out=sspk[:, NT + m:NT + m + 1],
                             in_=mscrs[m][:], axis=mybir.AxisListType.X)
                v.drain().then_inc(vsem, 1)                          # v6+m
            v.wait_ge(asem, 14)     # lnall ready
            # val = ln(exp(logit_pick)) - ln(sum exp) per row
            v.tensor_sub(val[:], lnall[:, NT:2 * NT], lnall[:, 0:NT])
            v.drain()
            v.wait_ge(smsk, 32)     # row-validity mask landed
            v.tensor_mul(vscr[:], val[:], rmask_sb[:])
            v.drain()
            v.reduce_sum(out=tmp[:], in_=vscr[:], axis=mybir.AxisListType.X)
            v.drain().then_inc(vsem, 1)                              # v10

        @block.scalar
        def _(a):
            a.wait_ge(vsem, 1)
            # Dummy exp: triggers the (single) activation table load during
            # the idx-DMA latency window instead of on the critical path.
            a.activation(out=scr1[:], in_=scr0[:], func=AF.Exp)
            a.drain().then_inc(asem, 1)                              # a1
            for t in range(NT):
                tile = slice(t * 128, (t + 1) * 128)
                a.wait_ge(sg[t], 16)
                # sum-of-squares per pixel comes free from the Square
                # activation's row accumulator
                a.activation(out=sqscr[:], in_=traw[:, tile], func=AF.Square,
                             accum_out=css[:, t:t + 1])
                a.drain()
                a.activation(out=lncss[:, t:t + 1], in_=css[:, t:t + 1],
                             func=AF.Ln)
                a.drain()
                # 1/norm = exp(-0.5 * ln(css))
                a.activation(out=inv[:, t:t + 1], in_=lncss[:, t:t + 1],
                             func=AF.Exp, scale=-0.5)
                a.drain().then_inc(asem, 1)                          # a2+t
            for t in range(NT):
                tile = slice(t * 128, (t + 1) * 128)
                a.wait_ge(psem, 1 + t)
                a.copy(out=rhsN[:, tile],
                       in_=Thats[t // 2][:, (t % 2) * 128:(t % 2 + 1) * 128])
                a.drain().then_inc(asem, 1)                          # a6+t
            if stage == "front":
                return
            for m in range(NT):
                a.wait_ge(psem, 5 + m)
                a.activation(out=escrs[m][:], in_=G0[m][:, 0:N], func=AF.Exp,
                             scale=INV_T, accum_out=sspk[:, m:m + 1])
                a.drain().then_inc(asem, 1)                          # a10+m
            if stage == "gram":
                a.dma_start(dbg_ssum_d[:], sspk[:, 0:NT]).then_inc(osem, 16)
                a.wait_ge(osem, 16)
                return
            a.wait_ge(vsem, 9)      # picked-exp reductions landed
            a.activation(out=lnall[:], in_=sspk[:], func=AF.Ln)
            a.drain().then_inc(asem, 1)                              # a14
            a.wait_ge(psem, 9)
            a.mul(res[:], tot[:], 1.0)
            a.drain().then_inc(asem, 1)                              # a15


        @block.tensor
        def _(te):
            te.wait_ge(sid, 16)     # identity landed
            for t in range(NT):
                tile = slice(t * 128, (t + 1) * 128)
                te.wait_ge(vsem, 2 + t)
                nc.tensor.transpose(
                    Thats[t // 2][:, (t % 2) * 128:(t % 2 + 1) * 128],
                    trawN[:, tile], I_sb[:])
                te.drain().then_inc(psem, 1)                         # p1+t
            if stage == "front":
                return
            te.wait_ge(asem, 9)     # all PSUM->SBUF copies done
            for m in range(NT):
                # float32r runs the PE at 1 cycle/row (vs 4 for fp32); the
                # tolerance (2e-2) dwarfs the precision delta.
                nc.tensor.matmul(
                    G0[m][:, 0:N], lhsT=rhsN[:, m * 128:(m + 1) * 128],
                    rhs=rhsN[:, 0:N], start=True, stop=True)
                te.drain().then_inc(psem, 1)                         # p5+m
            if stage == "gram":
                return
            te.wait_ge(vsem, 10)
            nc.tensor.matmul(tot[:], lhsT=tmp[:], rhs=ones_r[:],
                             start=True, stop=True)
            te.drain().then_inc(psem, 1)                             # p9

    return nc


_PROGS = {}


def _progs():
    if not _PROGS:
        _PROGS["main"] = _build_main_prog()
    return _PROGS


def _pixel_indices(gt_boxes: np.ndarray) -> np.ndarray:
    """Exact fp32 replication of the reference pixel-index math (last batch)."""
    boxes = np.asarray(gt_boxes)[B - 1].astype(np.float32, copy=False)
    x = boxes[:, 0].astype(np.float32)
    y = boxes[:, 1].astype(np.float32)
    span_w = PC_RANGE[3] - PC_RANGE[0]
    span_h = PC_RANGE[4] - PC_RANGE[1]
    cx = (x - np.float32(PC_RANGE[0])) / np.float32(span_w) * np.float32(W)
    cy = (y - np.float32(PC_RANGE[1])) / np.float32(span_h) * np.float32(H)
    cx = np.clip(cx.astype(np.int32), 0, W - 1)
    cy = np.clip(cy.astype(np.int32), 0, H - 1)
    return (cy.astype(np.int64) * W + cx.astype(np.int64)).astype(np.int32)


def kernel(spatial_features_2d, gt_boxes, static_labels, dynamic_labels,
           num_static=None, **_unused):
    sf = np.asarray(spatial_features_2d)
    pix = _pixel_indices(gt_boxes)  # [N] int32, linear index into H*W plane

    # Pixel-major table: one 512 B contiguous channel row per pixel.
    table = np.ascontiguousarray(
        sf[B - 1].reshape(C, HW).T, dtype=np.float32)   # [HW, C]

    pix_pad = np.zeros(NPAD, dtype=np.int32)
    pix_pad[:N] = pix
    idx = np.ascontiguousarray(pix_pad.reshape(NT, 128).T)  # [128, NT]

    labels = np.concatenate(
        [np.asarray(static_labels), np.asarray(dynamic_labels)], axis=0
    ).astype(np.int64)
    # One-hot label mask, row-block-major: block m in columns [m*N,(m+1)*N).
    lmask = np.zeros((128, NT * N), dtype=np.float32)
    rmask = np.zeros((128, NT), dtype=np.float32)
    for r in range(NT * 128):
        m, p = divmod(r, 128)
        if r < N:
            lmask[p, m * N + int(labels[r])] = 1.0
            rmask[p, m] = 1.0
        else:
            # invalid rows still need a finite ln(picked-exp); rmask zeroes
            # their contribution, but 0*ln(0) would be NaN
            lmask[p, m * N] = 1.0

    ident = np.eye(128, dtype=np.float32)

    in_maps = [{
        "table": table,
        "idx": idx,
        "ident": ident,
        "lmask": lmask,
        "rmask": rmask,
    }]
    r = _run("main", in_maps, core_ids=[0])
    total = float(r.results[0]["partial"][0, 0])
    _DEBUG["total"] = total
    loss = np.float32(total * (-LOSS_SCALE / N))
    return np.array(loss, dtype=np.float32)
